# revision 9
# baseline (speedup 1.0000x reference)
"""Trainium2 Bass kernel for a single-layer GRU (T=200, N=1024, H=128).

8 NeuronCores, data-parallel over batch (128 rows per core).

Layout strategy: all on-chip state is TRANSPOSED (hidden on the 128
partitions, batch on the free dim), so the recurrent matmuls use the
constant W_hh chunks as the stationary operand and h^T as the moving
operand — no per-step transpose of the state.  The host pre-transposes
x / h0 / weights into this layout when sharding (part of the data
distribution), and packs the biases per-partition.

Per step t:
    gh_rz accumulates onto the PSUM-resident gi_rz (input GEMM runs 2
    groups ahead, 4 steps per group, straight into PSUM banks):
        r = sigmoid(gh_r + gi_r + b_r)      ACT (bias = per-partition AP)
        z = sigmoid(gh_z + gi_z + b_z)      ACT
        rg = (gh_n + b_hhn) * r             DVE scalar_tensor_tensor
        q  = (gi_n + b_ihn) + rg            DVE scalar_tensor_tensor
        n  = tanh(q)                        ACT
        e  = h - n                          GPSIMD
        t  = z * e                          GPSIMD
        h' = n + t     (bf16, the state)    DVE
    h' is PE-transposed back to batch-major, copied PSUM->SBUF fp32 and
    DMA'd out in 4-step batches.
"""

import numpy as np
import ml_dtypes

import concourse.bass as bass
import concourse.bacc as bacc
import concourse.mybir as mybir
import concourse.tile as tile
from concourse.bass_utils import run_bass_kernel_spmd

F32 = mybir.dt.float32
BF16 = mybir.dt.bfloat16
AF = mybir.ActivationFunctionType
OP = mybir.AluOpType
BF = ml_dtypes.bfloat16

T_FULL = 200
N_FULL = 1024
H = 128
NCORES = 8
B = N_FULL // NCORES  # 128 batch rows per core
GS = 4  # steps per input-GEMM group


def build_gru(T: int) -> bass.Bass:
    assert T % GS == 0
    ngroups = T // GS
    nc = bacc.Bacc()

    # host-prepared, already transposed/packed
    xT_d = nc.declare_dram_parameter("xT", [H, T * B], BF16, isOutput=False)
    hxT_d = nc.declare_dram_parameter("hxT", [H, B], BF16, isOutput=False)
    whhT_d = nc.declare_dram_parameter("whhT", [H, 3 * H], BF16, isOutput=False)
    wihT_d = nc.declare_dram_parameter("wihT", [H, 3 * H], BF16, isOutput=False)
    bp_d = nc.declare_dram_parameter("bp", [128, 4], F32, isOutput=False)
    id_d = nc.declare_dram_parameter("idm", [128, 128], BF16, isOutput=False)
    out_d = nc.declare_dram_parameter("out", [T, B, H], F32, isOutput=True)
    hn_d = nc.declare_dram_parameter("h_n", [B, H], F32, isOutput=True)

    with tile.TileContext(nc) as tc:
        with (
            tc.tile_pool(name="consts", bufs=1) as consts,
            tc.tile_pool(name="xin", bufs=3) as xin,
            tc.tile_pool(name="work", bufs=3) as work,
            tc.tile_pool(name="hstate", bufs=3) as hstate,
            tc.tile_pool(name="outp", bufs=3) as outp,
            tc.tile_pool(name="psum", bufs=1, space="PSUM") as psum,
        ):
            # ---------------- one-time setup (simple contiguous DMAs) -----
            whhT = consts.tile([H, 3 * H], BF16)
            nc.sync.dma_start(out=whhT, in_=whhT_d[:])
            wihT = consts.tile([H, 3 * H], BF16)
            nc.sync.dma_start(out=wihT, in_=wihT_d[:])
            bp = consts.tile([128, 4], F32)
            nc.sync.dma_start(out=bp, in_=bp_d[:])
            idm = consts.tile([128, 128], BF16)
            nc.sync.dma_start(out=idm, in_=id_d[:])
            hb = consts.tile([H, B], BF16, name="h0b")
            nc.sync.dma_start(out=hb, in_=hxT_d[:])

            # warm the ACT function table before the hot loop so the
            # table-load pseudo-instruction doesn't ride on a hot sigmoid
            warm = consts.tile([128, 1], F32, name="warm")
            nc.scalar.activation(warm, bp[:, 0:1], AF.Sigmoid)
            nc.scalar.activation(warm, warm, AF.Tanh)
            warm2 = consts.tile([128, 4], F32, name="warm2")
            nc.vector.tensor_copy(warm2, bp)

            # rotating PSUM banks (allocated once, sliced manually)
            ghn_bank = psum.tile([128, 4 * B], F32, name="ghn_bank")
            tr_bank = psum.tile([128, 4 * B], BF16, name="tr_bank")

            gr_tiles: dict[int, bass.AP] = {}
            gz_tiles: dict[int, bass.AP] = {}
            gin_tiles: dict[int, bass.AP] = {}

            def emit_group(g: int):
                """x^T slice DMA + gi matmuls for steps [4g, 4g+4)."""
                t0 = g * GS
                xg = xin.tile([H, GS * B], BF16, name="xg", tag="xg")
                nc.sync.dma_start(out=xg, in_=xT_d[:, t0 * B:(t0 + GS) * B])
                gr = psum.tile([128, GS * B], F32, name="gr", tag="gr", bufs=2)
                gz = psum.tile([128, GS * B], F32, name="gz", tag="gz", bufs=2)
                gin = psum.tile([128, GS * B], F32, name="gin", tag="gin", bufs=2)
                nc.tensor.matmul(gr, wihT[:, 0:128], xg, start=True, stop=False,
                                 skip_group_check=True)
                nc.tensor.matmul(gz, wihT[:, 128:256], xg, start=True, stop=False,
                                 skip_group_check=True)
                nc.tensor.matmul(gin, wihT[:, 256:384], xg, start=True, stop=True)
                gr_tiles[g] = gr
                gz_tiles[g] = gz
                gin_tiles[g] = gin

            emit_group(0)
            if ngroups > 1:
                emit_group(1)

            out4 = None
            for t in range(T):
                g, j = divmod(t, GS)
                if j == 0 and g + 2 < ngroups:
                    emit_group(g + 2)
                sl = slice(j * B, (j + 1) * B)
                gr, gz, gin = gr_tiles[g], gz_tiles[g], gin_tiles[g]

                # recurrent matmuls accumulate onto the gi seeds
                nc.tensor.matmul(gr[:, sl], whhT[:, 0:128], hb, start=False,
                                 stop=True, skip_group_check=True)
                nc.tensor.matmul(gz[:, sl], whhT[:, 128:256], hb, start=False,
                                 stop=True, skip_group_check=True)
                ghn = ghn_bank[:, sl]
                nc.tensor.matmul(ghn, whhT[:, 256:384], hb, start=True, stop=True)

                r_sb = work.tile([128, B], F32, name="r_sb", tag="r_sb")
                nc.scalar.activation(r_sb, gr[:, sl], AF.Sigmoid, bias=bp[:, 0:1])
                z_sb = work.tile([128, B], F32, name="z_sb", tag="z_sb")
                nc.scalar.activation(z_sb, gz[:, sl], AF.Sigmoid, bias=bp[:, 1:2])

                rg_sb = work.tile([128, B], F32, name="rg_sb", tag="rg_sb")
                nc.vector.scalar_tensor_tensor(
                    out=rg_sb, in0=ghn, scalar=bp[:, 2:3], in1=r_sb,
                    op0=OP.add, op1=OP.mult,
                )
                q_sb = work.tile([128, B], F32, name="q_sb", tag="q_sb")
                nc.vector.scalar_tensor_tensor(
                    out=q_sb, in0=gin[:, sl], scalar=bp[:, 3:4], in1=rg_sb,
                    op0=OP.add, op1=OP.add,
                )
                n_sb = work.tile([128, B], F32, name="n_sb", tag="n_sb")
                nc.scalar.activation(n_sb, q_sb, AF.Tanh)

                e_sb = work.tile([128, B], F32, name="e_sb", tag="e_sb")
                nc.vector.tensor_sub(e_sb, hb, n_sb)
                t_sb = work.tile([128, B], F32, name="t_sb", tag="t_sb")
                nc.vector.tensor_mul(t_sb, z_sb, e_sb)

                h_new = hstate.tile([128, B], BF16, name="h_new", tag="h_new")
                nc.vector.tensor_add(h_new, n_sb, t_sb)
                hb = h_new

                # transpose back to batch-major for the output
                trp = tr_bank[:, sl]
                nc.tensor.transpose(trp, hb, idm)
                if j == 0:
                    out4 = outp.tile([128, GS, 128], F32, name="out4", tag="out4")
                nc.vector.tensor_copy(out4[:, j, :], trp)
                if j == GS - 1:
                    t0 = g * GS
                    nc.sync.dma_start(
                        out=out_d[t0:t0 + GS].rearrange("t b h -> b t h"),
                        in_=out4,
                    )
                if t == T - 1:
                    nc.sync.dma_start(out=hn_d[:], in_=out4[:, j, :])

    nc.finalize()
    return nc


_CACHE: dict = {}


def _get_nc(T: int) -> bass.Bass:
    if T not in _CACHE:
        _CACHE[T] = build_gru(T)
    return _CACHE[T]


def _prep_inmaps(input, hx, w_ih, w_hh, b_ih, b_hh):
    T = input.shape[0]
    whhT = np.ascontiguousarray(w_hh.T).astype(BF)  # (H, 3H)
    wihT = np.ascontiguousarray(w_ih.T).astype(BF)
    bsum = b_ih + b_hh
    bp = np.stack(
        [bsum[0:128], bsum[128:256], b_hh[256:384], b_ih[256:384]], axis=1
    ).astype(np.float32)  # (128, 4)
    idm = np.eye(128, dtype=np.float32).astype(BF)
    in_maps = []
    for i in range(NCORES):
        xs = input[:, i * B:(i + 1) * B, :]  # (T, B, H)
        xT = np.ascontiguousarray(xs.transpose(2, 0, 1).reshape(H, T * B)).astype(BF)
        hxT = np.ascontiguousarray(hx[0, i * B:(i + 1) * B, :].T).astype(BF)
        in_maps.append({
            "xT": xT,
            "hxT": hxT,
            "whhT": whhT,
            "wihT": wihT,
            "bp": bp,
            "idm": idm,
        })
    return in_maps


def _run(input, hx, w_ih, w_hh, b_ih, b_hh, trace=False):
    input = np.asarray(input, dtype=np.float32)
    T = input.shape[0]
    nc = _get_nc(T)
    hx = np.asarray(hx, dtype=np.float32)
    w_ih = np.asarray(w_ih, dtype=np.float32)
    w_hh = np.asarray(w_hh, dtype=np.float32)
    b_ih = np.asarray(b_ih, dtype=np.float32)
    b_hh = np.asarray(b_hh, dtype=np.float32)

    in_maps = _prep_inmaps(input, hx, w_ih, w_hh, b_ih, b_hh)
    res = run_bass_kernel_spmd(nc, in_maps, core_ids=list(range(NCORES)),
                               trace=trace)
    outs = np.concatenate([res.results[i]["out"] for i in range(NCORES)], axis=1)
    hn = np.concatenate([res.results[i]["h_n"] for i in range(NCORES)], axis=0)[None]
    return (outs.astype(np.float32), hn.astype(np.float32)), res


def kernel(input, hx, w_ih, w_hh, b_ih, b_hh):
    (outs, hn), _ = _run(input, hx, w_ih, w_hh, b_ih, b_hh, trace=False)
    return outs, hn


# ---------------- timing utilities (test-only, not used by kernel()) ------

def _make_runner(nc, in_maps):
    """Replicates bass2jax.run_bass_via_pjrt but returns a reusable jitted
    callable + device-resident inputs so repeated executions can be timed."""
    import jax
    import jax.numpy as jnp
    from jax.sharding import Mesh, PartitionSpec
    from jax.experimental.shard_map import shard_map
    import concourse.bass2jax as b2j
    import concourse.mybir as mybir

    b2j.install_neuronx_cc_hook()
    n_cores = len(in_maps)
    partition_name = nc.partition_id_tensor.name if nc.partition_id_tensor else None
    in_names, out_names, out_avals, zero_outs = [], [], [], []
    for alloc in nc.m.functions[0].allocations:
        if not isinstance(alloc, mybir.MemoryLocationSet):
            continue
        name = alloc.memorylocations[0].name
        if alloc.kind == "ExternalInput":
            if name != partition_name:
                in_names.append(name)
        elif alloc.kind == "ExternalOutput":
            out_names.append(name)
            shape = tuple(alloc.tensor_shape)
            dtype = mybir.dt.np(alloc.dtype)
            out_avals.append(jax.core.ShapedArray(shape, dtype))
            zero_outs.append(np.zeros(shape, dtype))
    n_params = len(in_names)
    n_outs = len(out_avals)
    in_names_all = in_names + out_names
    if partition_name is not None:
        in_names_all.append(partition_name)

    def _body(*args):
        operands = list(args)
        if partition_name is not None:
            operands.append(b2j.partition_id_tensor())
        outs = b2j._bass_exec_p.bind(
            *operands,
            out_avals=tuple(out_avals),
            in_names=tuple(in_names_all),
            out_names=tuple(out_names),
            lowering_input_output_aliases=(),
            sim_require_finite=True,
            sim_require_nnan=True,
            nc=nc,
        )
        return tuple(outs)

    donate = tuple(range(n_params, n_params + n_outs))
    devices = jax.devices()[:n_cores]
    mesh = Mesh(np.asarray(devices), ("core",))
    in_specs = (PartitionSpec("core"),) * (n_params + n_outs)
    out_specs = (PartitionSpec("core"),) * len(out_names)
    sharded = jax.jit(
        shard_map(_body, mesh=mesh, in_specs=in_specs, out_specs=out_specs,
                  check_rep=False),
        donate_argnums=donate, keep_unused=True,
    )
    concat_in = [
        np.concatenate([np.asarray(in_maps[c][name]) for c in range(n_cores)], axis=0)
        for name in in_names
    ]
    sharding = jax.sharding.NamedSharding(mesh, PartitionSpec("core"))
    dev_in = [jax.device_put(a, sharding) for a in concat_in]
    zero_shapes = [(n_cores * z.shape[0], *z.shape[1:]) for z in zero_outs]
    zero_dtypes = [z.dtype for z in zero_outs]

    def make_zeros():
        return [jax.device_put(jnp.zeros(s, d), sharding)
                for s, d in zip(zero_shapes, zero_dtypes)]

    def run():
        outs = sharded(*dev_in, *make_zeros())
        jax.block_until_ready(outs)
        return outs

    return run


def bench(input, hx, w_ih, w_hh, b_ih, b_hh, iters=20):
    """Returns (best_wall_s, [all walls]) for the full SPMD execution,
    device-resident inputs, after one warmup."""
    import time
    input = np.asarray(input, dtype=np.float32)
    T = input.shape[0]
    nc = _get_nc(T)
    hx = np.asarray(hx, dtype=np.float32)
    in_maps = _prep_inmaps(input, hx, np.asarray(w_ih, np.float32),
                           np.asarray(w_hh, np.float32),
                           np.asarray(b_ih, np.float32),
                           np.asarray(b_hh, np.float32))
    run = _make_runner(nc, in_maps)
    run()  # warmup + compile
    walls = []
    for _ in range(iters):
        t0 = time.perf_counter()
        run()
        walls.append(time.perf_counter() - t0)
    return min(walls), walls


# revision 25
# speedup vs baseline: 1.0406x; 1.0406x over previous
"""Trainium2 Bass kernel for a single-layer GRU (T=200, N=1024, H=128).

8 NeuronCores, data-parallel over batch (128 rows per core).

Layout strategy: all on-chip state is TRANSPOSED (hidden on the 128
partitions, batch on the free dim), so the recurrent matmuls use the
constant W_hh chunks as the stationary operand and h^T as the moving
operand — no per-step transpose of the state.  The host pre-transposes
x / h0 / weights into this layout when sharding (part of the data
distribution), and packs the biases per-partition.

Per step t:
    gh_rz accumulates onto the PSUM-resident gi_rz (input GEMM runs 2
    groups ahead, 4 steps per group, straight into PSUM banks):
        r = sigmoid(gh_r + gi_r + b_r)      ACT (bias = per-partition AP)
        z = sigmoid(gh_z + gi_z + b_z)      ACT
        rg = (gh_n + b_hhn) * r             DVE scalar_tensor_tensor
        q  = (gi_n + b_ihn) + rg            DVE scalar_tensor_tensor
        n  = tanh(q)                        ACT
        e  = h - n                          GPSIMD
        t  = z * e                          GPSIMD
        h' = n + t     (bf16, the state)    DVE
    h' is PE-transposed back to batch-major, copied PSUM->SBUF fp32 and
    DMA'd out in 4-step batches.
"""

import numpy as np
import ml_dtypes

import concourse.bass as bass
import concourse.bacc as bacc
import concourse.mybir as mybir
import concourse.tile as tile
from concourse.bass_utils import run_bass_kernel_spmd

F32 = mybir.dt.float32
BF16 = mybir.dt.bfloat16
AF = mybir.ActivationFunctionType
OP = mybir.AluOpType
BF = ml_dtypes.bfloat16

T_FULL = 200
N_FULL = 1024
H = 128
NCORES = 8
B = N_FULL // NCORES  # 128 batch rows per core
GS = 4  # steps per input-GEMM group


def build_gru(T: int) -> bass.Bass:
    assert T % GS == 0
    ngroups = T // GS
    nc = bacc.Bacc()

    # host-prepared, already transposed/packed
    xT_d = nc.declare_dram_parameter("xT", [H, T * B], BF16, isOutput=False)
    hxT_d = nc.declare_dram_parameter("hxT", [H, B], BF16, isOutput=False)
    whhT_d = nc.declare_dram_parameter("whhT", [H, 3 * H], BF16, isOutput=False)
    wihT_d = nc.declare_dram_parameter("wihT", [H, 3 * H], BF16, isOutput=False)
    bp_d = nc.declare_dram_parameter("bp", [128, 4], F32, isOutput=False)
    brows_d = nc.declare_dram_parameter("brows", [2, 128], BF16, isOutput=False)
    mask_d = nc.declare_dram_parameter("mask", [2, 512], BF16, isOutput=False)
    ones_d = nc.declare_dram_parameter("ones", [1, 512], BF16, isOutput=False)
    bn_d = nc.declare_dram_parameter("bnrow", [1, 128], BF16, isOutput=False)
    id32_d = nc.declare_dram_parameter("id32", [128, 128], F32, isOutput=False)
    id_d = nc.declare_dram_parameter("idm", [128, 128], BF16, isOutput=False)
    out_d = nc.declare_dram_parameter("out", [T, B, H], F32, isOutput=True)
    hn_d = nc.declare_dram_parameter("h_n", [B, H], F32, isOutput=True)

    with tile.TileContext(nc) as tc:
        with (
            tc.tile_pool(name="consts", bufs=1) as consts,
            tc.tile_pool(name="xin", bufs=3) as xin,
            tc.tile_pool(name="work", bufs=4) as work,
            tc.tile_pool(name="hstate", bufs=3) as hstate,
            tc.tile_pool(name="outp", bufs=3) as outp,
            tc.tile_pool(name="psum", bufs=1, space="PSUM") as psum,
        ):
            C = 2           # independent chains (batch split)
            BC = B // C     # 64 batch rows per chain
            # ---------------- one-time setup (simple contiguous DMAs) -----
            whhT = consts.tile([H, 3 * H], BF16)
            nc.sync.dma_start(out=whhT, in_=whhT_d[:])
            wihT = consts.tile([H, 3 * H], BF16)
            nc.sync.dma_start(out=wihT, in_=wihT_d[:])
            bp = consts.tile([128, 4], F32)
            nc.sync.dma_start(out=bp, in_=bp_d[:])
            brz = consts.tile([2, 128], BF16)
            nc.sync.dma_start(out=brz, in_=brows_d[:])
            mask = consts.tile([2, 512], BF16)
            nc.sync.dma_start(out=mask, in_=mask_d[:])
            ones = consts.tile([1, 512], BF16)
            nc.sync.dma_start(out=ones, in_=ones_d[:])
            bnrow = consts.tile([1, 128], BF16)
            nc.sync.dma_start(out=bnrow, in_=bn_d[:])
            id32 = consts.tile([128, 128], F32)
            nc.sync.dma_start(out=id32, in_=id32_d[:])
            idm = consts.tile([128, 128], BF16)
            nc.sync.dma_start(out=idm, in_=id_d[:])
            hb0 = consts.tile([H, B], BF16, name="h0b")
            nc.sync.dma_start(out=hb0, in_=hxT_d[:])

            # warm the ACT function table before the hot loop
            warm = consts.tile([128, 1], F32, name="warm")
            nc.scalar.activation(warm, bp[:, 0:1], AF.Sigmoid)
            nc.scalar.activation(warm, warm, AF.Tanh)
            warm2 = consts.tile([128, 4], F32, name="warm2")
            nc.vector.tensor_copy(warm2, bp)

            # rotating PSUM banks (allocated once, sliced manually)
            # ghn: slot per (j=t%4, c): (j*2+c)*BC, width BC
            ghn_bank = psum.tile([128, 4 * 2 * BC], F32, name="ghn_bank")
            # tr: slot per j=t%4: (128, 128); chains write partition halves
            tr_bank = psum.tile([128, 4, 128], BF16, name="tr_bank")

            # grz group tile: 8 blocks of 128 cols; block k = j*2+c holds
            # [r(BC) | z(BC)] for step 4g+j chain c  -> 1024 cols = 2 banks
            # gin group tile: block (j,c) at (j*2+c)*BC -> 512 cols = 1 bank
            grz_tiles: dict[int, bass.AP] = {}
            gin_tiles: dict[int, bass.AP] = {}

            pending_mms: list = []

            def emit_group(g: int):
                t0 = g * GS
                xg = xin.tile([H, GS * B], BF16, name="xg", tag="xg")
                nc.sync.dma_start(out=xg, in_=xT_d[:, t0 * B:(t0 + GS) * B])
                # per-chain grz bank: [r: 4j x 64 | z: 4j x 64]
                grzs = []
                xg4 = xg.rearrange("h (j b) -> h j b", j=GS)
                gin = psum.tile([128, 8, BC], F32, name="gin", tag="gin", bufs=2)
                gin4 = gin.rearrange("p (j c) b -> p j c b", c=2)
                nc.tensor.matmul(gin.rearrange("p j b -> p (j b)"), bnrow, ones,
                                 start=True, stop=False, skip_group_check=True)
                for c in range(C):
                    gc = psum.tile([128, 512], F32, name=f"grz{c}", tag=f"grz{c}",
                                   bufs=2)
                    nc.tensor.matmul(gc, brz, mask, start=True, stop=False,
                                     skip_group_check=True)
                    grzs.append(gc)
                # defer the 24 gi matmuls; drained a few per step so they
                # never monopolize the PE queue ahead of recurrent matmuls
                def emit_gi(rg_i, wsl, c, j):
                    rhs = xg4[:, j, c * BC:(c + 1) * BC]
                    if rg_i < 2:
                        nc.tensor.matmul(
                            grzs[c][:, rg_i * 256 + j * BC: rg_i * 256 + (j + 1) * BC],
                            wihT[:, wsl], rhs,
                            start=False, stop=False, skip_group_check=True)
                    else:
                        nc.tensor.matmul(gin4[:, j, c, :], wihT[:, 256:384], rhs,
                                         start=False, stop=False,
                                         skip_group_check=True)
                for rg_i, wsl in ((0, slice(0, 128)), (1, slice(128, 256)),
                                  (2, slice(256, 384))):
                    for c in range(C):
                        for j in range(GS):
                            pending_mms.append((emit_gi, rg_i, wsl, c, j))
                grz_tiles[g] = grzs
                gin_tiles[g] = gin

            def drain_pending(k):
                for _ in range(min(k, len(pending_mms))):
                    f, *args = pending_mms.pop(0)
                    f(*args)

            emit_group(0)
            if ngroups > 1:
                emit_group(1)

            hb = [hb0[:, c * BC:(c + 1) * BC] for c in range(C)]
            out4 = None

            def mm_phase(t, c):
                g, j = divmod(t, GS)
                gc = grz_tiles[g][c]
                k = j * 2 + c
                nc.tensor.matmul(gc[:, j * BC:(j + 1) * BC], whhT[:, 0:128], hb[c],
                                 start=False, stop=False, skip_group_check=True)
                nc.tensor.matmul(gc[:, 256 + j * BC:256 + (j + 1) * BC],
                                 whhT[:, 128:256], hb[c],
                                 start=False, stop=(j == GS - 1),
                                 skip_group_check=True)
                nc.tensor.matmul(ghn_bank[:, k * BC:(k + 1) * BC],
                                 whhT[:, 256:384], hb[c], start=True, stop=True)

            def sig_phase(t, c):
                g, j = divmod(t, GS)
                g3 = grz_tiles[g][c].rearrange("p (rg jb) -> p rg jb", rg=2)
                rz = work.tile([128, 2, BC], BF16, name=f"rz{c}", tag=f"rz{c}")
                nc.scalar.activation(rz, g3[:, :, j * BC:(j + 1) * BC], AF.Sigmoid)
                return rz.rearrange("p r b -> p (r b)")

            def rgq_phase(t, c, rz):
                g, j = divmod(t, GS)
                k = j * 2 + c
                rg = work.tile([128, BC], F32, name=f"rg{c}", tag=f"rg{c}")
                nc.vector.scalar_tensor_tensor(
                    out=rg, in0=ghn_bank[:, k * BC:(k + 1) * BC],
                    scalar=bp[:, 2:3], in1=rz[:, 0:BC], op0=OP.add, op1=OP.mult)
                q = work.tile([128, BC], F32, name=f"q{c}", tag=f"q{c}")
                nc.vector.scalar_tensor_tensor(
                    out=q, in0=gin_tiles[g][:, k, :],
                    scalar=0.0, in1=rg, op0=OP.add, op1=OP.add)
                return q

            def tanh_phase(t, c, q):
                n = work.tile([128, BC], F32, name=f"n{c}", tag=f"n{c}")
                nc.scalar.activation(n, q, AF.Tanh)
                return n

            def upd_phase(t, c, rz, n):
                e = work.tile([128, BC], F32, name=f"e{c}", tag=f"e{c}")
                nc.gpsimd.tensor_sub(e, hb[c], n)
                tt = work.tile([128, BC], F32, name=f"tt{c}", tag=f"tt{c}")
                nc.gpsimd.tensor_mul(tt, rz[:, BC:128], e)
                h_new = hstate.tile([128, BC], BF16, name=f"h{c}", tag=f"h{c}")
                nc.gpsimd.tensor_add(h_new, n, tt)
                hb[c] = h_new

            for t in range(T):
                g, j = divmod(t, GS)
                if j == 0 and g + 2 < ngroups:
                    emit_group(g + 2)
                if j == 0:
                    # everything for group g+1 must be emitted by now
                    drain_pending(len(pending_mms) - 24 if g + 2 < ngroups else
                                  len(pending_mms))

                mm_phase(t, 0)
                mm_phase(t, 1)
                rz0 = sig_phase(t, 0)
                q0 = rgq_phase(t, 0, rz0)
                rz1 = sig_phase(t, 1)
                drain_pending(3)
                n0 = tanh_phase(t, 0, q0)
                q1 = rgq_phase(t, 1, rz1)
                upd_phase(t, 0, rz0, n0)
                drain_pending(3)
                n1 = tanh_phase(t, 1, q1)
                upd_phase(t, 1, rz1, n1)

                # output: transpose both chains into one (128,128) slot
                nc.tensor.transpose(tr_bank[0:BC, j, :], hb[0], idm)
                nc.tensor.transpose(tr_bank[BC:128, j, :], hb[1], idm,
                                    tile_position=(0, BC))
                if j == GS - 1:
                    t0 = g * GS
                    out4 = outp.tile([128, GS, 128], F32, name="out4", tag="out4")
                    nc.vector.tensor_copy(out4, tr_bank)
                    nc.sync.dma_start(
                        out=out_d[t0:t0 + GS].rearrange("t b h -> b t h"),
                        in_=out4,
                    )
                    if t == T - 1:
                        nc.sync.dma_start(out=hn_d[:], in_=out4[:, j, :])

    nc.finalize()
    return nc


_CACHE: dict = {}


def _get_nc(T: int) -> bass.Bass:
    if T not in _CACHE:
        _CACHE[T] = build_gru(T)
    return _CACHE[T]


def _prep_inmaps(input, hx, w_ih, w_hh, b_ih, b_hh):
    T = input.shape[0]
    whhT = np.ascontiguousarray(w_hh.T).astype(BF)  # (H, 3H)
    wihT = np.ascontiguousarray(w_ih.T).astype(BF)
    bsum = b_ih + b_hh
    bp = np.stack(
        [bsum[0:128], bsum[128:256], b_hh[256:384], b_ih[256:384]], axis=1
    ).astype(np.float32)  # (128, 4)
    idm = np.eye(128, dtype=np.float32).astype(BF)
    in_maps = []
    for i in range(NCORES):
        xs = input[:, i * B:(i + 1) * B, :]  # (T, B, H)
        xT = np.ascontiguousarray(xs.transpose(2, 0, 1).reshape(H, T * B)).astype(BF)
        hxT = np.ascontiguousarray(hx[0, i * B:(i + 1) * B, :].T).astype(BF)
        in_maps.append({
            "xT": xT,
            "hxT": hxT,
            "whhT": whhT,
            "wihT": wihT,
            "bp": bp,
            "brows": np.stack([bsum[0:128], bsum[128:256]]).astype(BF),
            "ones": np.ones((1, 512), np.float32).astype(BF),
            "bnrow": b_ih[None, 256:384].astype(BF),
            "id32": np.eye(128, dtype=np.float32),
            "mask": np.stack([
                np.r_[np.ones(256), np.zeros(256)],
                np.r_[np.zeros(256), np.ones(256)]]
            ).astype(np.float32).astype(BF),
            "idm": idm,
        })
    return in_maps


def _run(input, hx, w_ih, w_hh, b_ih, b_hh, trace=False):
    input = np.asarray(input, dtype=np.float32)
    T = input.shape[0]
    nc = _get_nc(T)
    hx = np.asarray(hx, dtype=np.float32)
    w_ih = np.asarray(w_ih, dtype=np.float32)
    w_hh = np.asarray(w_hh, dtype=np.float32)
    b_ih = np.asarray(b_ih, dtype=np.float32)
    b_hh = np.asarray(b_hh, dtype=np.float32)

    in_maps = _prep_inmaps(input, hx, w_ih, w_hh, b_ih, b_hh)
    res = run_bass_kernel_spmd(nc, in_maps, core_ids=list(range(NCORES)),
                               trace=trace)
    outs = np.concatenate([res.results[i]["out"] for i in range(NCORES)], axis=1)
    hn = np.concatenate([res.results[i]["h_n"] for i in range(NCORES)], axis=0)[None]
    return (outs.astype(np.float32), hn.astype(np.float32)), res


def kernel(input, hx, w_ih, w_hh, b_ih, b_hh):
    (outs, hn), _ = _run(input, hx, w_ih, w_hh, b_ih, b_hh, trace=False)
    return outs, hn


# ---------------- timing utilities (test-only, not used by kernel()) ------

def _make_runner(nc, in_maps):
    """Replicates bass2jax.run_bass_via_pjrt but returns a reusable jitted
    callable + device-resident inputs so repeated executions can be timed."""
    import jax
    import jax.numpy as jnp
    from jax.sharding import Mesh, PartitionSpec
    from jax.experimental.shard_map import shard_map
    import concourse.bass2jax as b2j
    import concourse.mybir as mybir

    b2j.install_neuronx_cc_hook()
    n_cores = len(in_maps)
    partition_name = nc.partition_id_tensor.name if nc.partition_id_tensor else None
    in_names, out_names, out_avals, zero_outs = [], [], [], []
    for alloc in nc.m.functions[0].allocations:
        if not isinstance(alloc, mybir.MemoryLocationSet):
            continue
        name = alloc.memorylocations[0].name
        if alloc.kind == "ExternalInput":
            if name != partition_name:
                in_names.append(name)
        elif alloc.kind == "ExternalOutput":
            out_names.append(name)
            shape = tuple(alloc.tensor_shape)
            dtype = mybir.dt.np(alloc.dtype)
            out_avals.append(jax.core.ShapedArray(shape, dtype))
            zero_outs.append(np.zeros(shape, dtype))
    n_params = len(in_names)
    n_outs = len(out_avals)
    in_names_all = in_names + out_names
    if partition_name is not None:
        in_names_all.append(partition_name)

    def _body(*args):
        operands = list(args)
        if partition_name is not None:
            operands.append(b2j.partition_id_tensor())
        outs = b2j._bass_exec_p.bind(
            *operands,
            out_avals=tuple(out_avals),
            in_names=tuple(in_names_all),
            out_names=tuple(out_names),
            lowering_input_output_aliases=(),
            sim_require_finite=True,
            sim_require_nnan=True,
            nc=nc,
        )
        return tuple(outs)

    donate = tuple(range(n_params, n_params + n_outs))
    devices = jax.devices()[:n_cores]
    mesh = Mesh(np.asarray(devices), ("core",))
    in_specs = (PartitionSpec("core"),) * (n_params + n_outs)
    out_specs = (PartitionSpec("core"),) * len(out_names)
    sharded = jax.jit(
        shard_map(_body, mesh=mesh, in_specs=in_specs, out_specs=out_specs,
                  check_rep=False),
        donate_argnums=donate, keep_unused=True,
    )
    concat_in = [
        np.concatenate([np.asarray(in_maps[c][name]) for c in range(n_cores)], axis=0)
        for name in in_names
    ]
    sharding = jax.sharding.NamedSharding(mesh, PartitionSpec("core"))
    dev_in = [jax.device_put(a, sharding) for a in concat_in]
    zero_shapes = [(n_cores * z.shape[0], *z.shape[1:]) for z in zero_outs]
    zero_dtypes = [z.dtype for z in zero_outs]

    def make_zeros():
        return [jax.device_put(jnp.zeros(s, d), sharding)
                for s, d in zip(zero_shapes, zero_dtypes)]

    def dispatch(zeros=None):
        return sharded(*dev_in, *(zeros if zeros is not None else make_zeros()))

    def run():
        outs = dispatch()
        jax.block_until_ready(outs)
        return outs

    run.dispatch = dispatch
    run.make_zeros = make_zeros
    return run


def bench(input, hx, w_ih, w_hh, b_ih, b_hh, iters=20):
    """Returns (best_wall_s, [all walls]) for the full SPMD execution,
    device-resident inputs, after one warmup."""
    import time
    input = np.asarray(input, dtype=np.float32)
    T = input.shape[0]
    nc = _get_nc(T)
    hx = np.asarray(hx, dtype=np.float32)
    in_maps = _prep_inmaps(input, hx, np.asarray(w_ih, np.float32),
                           np.asarray(w_hh, np.float32),
                           np.asarray(b_ih, np.float32),
                           np.asarray(b_hh, np.float32))
    run = _make_runner(nc, in_maps)
    run()  # warmup + compile
    walls = []
    for _ in range(iters):
        t0 = time.perf_counter()
        run()
        walls.append(time.perf_counter() - t0)
    return min(walls), walls
